# revision 9
# baseline (speedup 1.0000x reference)
"""Trainium2 Bass kernel for nn_AlainFC: deep narrow MLP, pure data parallel.

Reference computation (B=16384):
  x [B,1,28,28] -> flat [B,784]
  first = relu(x @ W1.T + b1)              # [B,128]
  h1 = 128x chain: h = relu(h @ Ws1[i].T + bs1[i])
  h2 = 8x chain on (first + h1) with Ws2/bs2
  out = h2 @ Wout.T + bout                 # [B,10]

Strategy: shard batch across 8 cores (2048 each), replicate weights.
Per core: activations live as [128 feat partitions, batch free] in SBUF.
Batch is split into 4 groups of 512 (one PSUM bank per matmul). bf16
matmuls (fp32 PSUM accumulate); bias+relu split across ScalarE (ACT) and
VectorE (DVE) per group so the PSUM drain is parallelized. All weights are
pre-transposed/packed/bf16-converted on host (free).
"""
import sys
import types

sys.path.insert(0, "/opt/trn_rl_repo")

import numpy as np
import ml_dtypes

N_CORES = 8
B = 16384
BC = B // N_CORES        # 2048 batch per core
H = 128
IN = 784
KT = 7                   # k-tiles for layer 0 (784 = 7*112)
KP = 112                 # partitions per k-tile
NG = 4                   # batch groups per core
GW = BC // NG            # 512 = group width (one PSUM bank)
NL = 136                 # chained 128x128 layers (128 + 8)
NCH = 17                 # weight DMA chunks
CHL = NL // NCH          # 8 layers per chunk
NC_OUT = 10

_BF16 = ml_dtypes.bfloat16
_COMPILED = None


def _ensure_axon_hooks():
    """Shim antenv.axon_hooks (missing on this image) so trace=True works."""
    try:
        import antenv
    except ImportError:
        return
    if "antenv.axon_hooks" in sys.modules:
        return
    mod = types.ModuleType("antenv.axon_hooks")
    mod._hook = None

    def set_axon_ntff_profile_hook(h):
        mod._hook = h

    def get_axon_ntff_profile_hook():
        return mod._hook

    mod.set_axon_ntff_profile_hook = set_axon_ntff_profile_hook
    mod.get_axon_ntff_profile_hook = get_axon_ntff_profile_hook
    sys.modules["antenv.axon_hooks"] = mod
    antenv.axon_hooks = mod
    try:
        from trn_agent_boot.trn_boot import _ntff_profile_via_ctypes

        hook = _ntff_profile_via_ctypes("/opt/axon/libaxon_pjrt.so")
        if hook is not None:
            set_axon_ntff_profile_hook(hook)
    except Exception:
        pass


def _build():
    import concourse.bass as bass
    import concourse.bacc as bacc
    import concourse.mybir as mybir
    from concourse import tile
    from concourse.tile_rust import add_dep_helper

    dt = mybir.dt
    AF = mybir.ActivationFunctionType
    ALU = mybir.AluOpType

    nc = bacc.Bacc("TRN2", target_bir_lowering=False, debug=False,
                   num_devices=N_CORES)

    xg_ext = nc.dram_tensor("xg", [NG, KP, KT * GW], dt.bfloat16,
                            kind="ExternalInput")
    w1g_ext = nc.dram_tensor("w1g", [KP, KT * H], dt.bfloat16,
                             kind="ExternalInput")
    wsg_ext = nc.dram_tensor("wsg", [NCH, H, CHL * H], dt.bfloat16,
                             kind="ExternalInput")
    bias_ext = nc.dram_tensor("bias", [H, NL + 1], dt.float32,
                              kind="ExternalInput")
    woutt_ext = nc.dram_tensor("woutt", [H, NC_OUT], dt.bfloat16,
                               kind="ExternalInput")
    bout_ext = nc.dram_tensor("bout", [NC_OUT, 1], dt.float32,
                              kind="ExternalInput")
    out_ext = nc.dram_tensor("out", [NC_OUT, BC], dt.float32,
                             kind="ExternalOutput")

    with tile.TileContext(nc) as tc:
        from contextlib import ExitStack

        with ExitStack() as ctx:
            const_pool = ctx.enter_context(tc.tile_pool(name="const", bufs=1))
            xg_pool = ctx.enter_context(tc.tile_pool(name="xg", bufs=NG))
            wsg_pool = ctx.enter_context(tc.tile_pool(name="wsg", bufs=NCH))
            first_pool = ctx.enter_context(tc.tile_pool(name="first", bufs=NG))
            h_pool = ctx.enter_context(tc.tile_pool(name="h", bufs=14))
            out_pool = ctx.enter_context(tc.tile_pool(name="osb", bufs=NG))
            psum_pool = ctx.enter_context(
                tc.tile_pool(name="psum", bufs=8, space="PSUM"))

            # ---- DMA inputs, STAGED: later stages wait on earlier
            # transfers so the group-0 chain inputs land first (engines
            # start ~10us earlier) instead of fair-sharing bandwidth ----
            prev_stage = []

            def staged(dma_instr):
                for p in prev_stage:
                    add_dep_helper(dma_instr.ins, p.ins, sync=True,
                                   reason="dma arrival staging")
                return dma_instr

            def stage_done(instrs):
                prev_stage.clear()
                prev_stage.extend(instrs)

            # stage 0: layer-0 critical path for group 0 (+ tiny consts)
            w1g_sb = const_pool.tile([KP, KT * H], dt.bfloat16, tag="w1g")
            s = [nc.sync.dma_start(w1g_sb[:], w1g_ext.ap())]
            bias_sb = const_pool.tile([H, NL + 1], dt.float32, tag="bias")
            s.append(nc.sync.dma_start(bias_sb[:], bias_ext.ap()))
            xg_sb = []
            QP = KP // 4
            for g in range(NG):
                t = xg_pool.tile([KP, KT * GW], dt.bfloat16, tag="xg")
                xg_sb.append(t)
            for q in range(4):
                s.append(nc.sync.dma_start(
                    xg_sb[0][q * QP:(q + 1) * QP, :],
                    xg_ext.ap()[0, q * QP:(q + 1) * QP]))
            stage_done(s)

            wsg_sb = []

            def dma_wsg(c):
                t = wsg_pool.tile([H, CHL * H], dt.bfloat16, tag="wsg")
                ins = staged(nc.sync.dma_start(t[:], wsg_ext.ap()[c]))
                wsg_sb.append(t)
                return ins

            def dma_xg(g):
                out = []
                for q in range(4):
                    out.append(staged(nc.sync.dma_start(
                        xg_sb[g][q * QP:(q + 1) * QP, :],
                        xg_ext.ap()[g, q * QP:(q + 1) * QP])))
                return out

            stage_done([dma_wsg(0)])
            stage_done(dma_xg(1))
            stage_done([dma_wsg(1), dma_wsg(2)])
            stage_done(dma_xg(2))
            stage_done([dma_wsg(3), dma_wsg(4)])
            stage_done(dma_xg(3))
            tail = [dma_wsg(c) for c in range(5, NCH)]
            woutt_sb = const_pool.tile([H, NC_OUT], dt.bfloat16, tag="woutt")
            tail.append(staged(nc.sync.dma_start(woutt_sb[:],
                                                 woutt_ext.ap())))
            bout_sb = const_pool.tile([NC_OUT, 1], dt.float32, tag="bout")
            tail.append(staged(nc.sync.dma_start(bout_sb[:], bout_ext.ap())))

            def relu_bias(g, psum_t, out_t, b_ap):
                # bias+relu: even groups on ScalarE, odd groups on VectorE
                # (one PSUM bank each -- parallel drain, no bank conflicts)
                if g % 2 == 0:
                    nc.scalar.activation(out_t[:], psum_t[:], AF.Relu,
                                         bias=b_ap)
                else:
                    nc.vector.tensor_scalar(out_t[:], psum_t[:], b_ap, 0.0,
                                            ALU.add, ALU.max)

            # ---- layer 0: [784 -> 128], K accumulated over 7 k-tiles ----
            h_cur = []
            for g in range(NG):
                ps = psum_pool.tile([H, GW], dt.float32, tag="ps")
                for k in range(KT):
                    nc.tensor.matmul(
                        ps[:],
                        w1g_sb[:, k * H:(k + 1) * H],
                        xg_sb[g][:, k * GW:(k + 1) * GW],
                        start=(k == 0), stop=(k == KT - 1),
                    )
                f = first_pool.tile([H, GW], dt.bfloat16, tag="first")
                relu_bias(g, ps, f, bias_sb[:, 0:1])
                h_cur.append(f)
            first_tiles = list(h_cur)

            # ---- 136 chained layers ----
            for l in range(NL):
                c, j = divmod(l, CHL)
                w_ap = wsg_sb[c][:, j * H:(j + 1) * H]
                b_ap = bias_sb[:, l + 1:l + 2]
                h_next = []
                for g in range(NG):
                    ps = psum_pool.tile([H, GW], dt.float32, tag="ps")
                    if l == 128:
                        # first Ws2 layer reads (first + h1): accumulate two
                        # matmuls instead of an explicit elementwise add
                        nc.tensor.matmul(ps[:], w_ap, h_cur[g][:],
                                         start=True, stop=False)
                        nc.tensor.matmul(ps[:], w_ap, first_tiles[g][:],
                                         start=False, stop=True)
                    else:
                        nc.tensor.matmul(ps[:], w_ap, h_cur[g][:],
                                         start=True, stop=True)
                    hn = h_pool.tile([H, GW], dt.bfloat16, tag="h")
                    relu_bias(g, ps, hn, b_ap)
                    h_next.append(hn)
                h_cur = h_next

            # ---- output layer: [128 -> 10] + bias (no relu) ----
            for g in range(NG):
                ps = psum_pool.tile([NC_OUT, GW], dt.float32, tag="ps")
                nc.tensor.matmul(ps[:], woutt_sb[:], h_cur[g][:],
                                 start=True, stop=True)
                ot = out_pool.tile([NC_OUT, GW], dt.float32, tag="ot")
                if g % 2 == 0:
                    nc.scalar.activation(ot[:], ps[:], AF.Identity,
                                         bias=bout_sb[:])
                else:
                    nc.vector.tensor_scalar(ot[:], ps[:], bout_sb[:], None,
                                            ALU.add)
                nc.sync.dma_start(out_ext.ap()[:, g * GW:(g + 1) * GW], ot[:])

    nc.compile()
    return nc


def _prep_inputs(x, W1, b1, Ws1, bs1, Ws2, bs2, Wout, bout):
    """Host-side shard + pack. Returns per-core input maps."""
    xf = np.ascontiguousarray(x.reshape(B, IN).T)          # [784, B]
    # per-core [784, 2048] -> groups [4, 112, 7*512]
    w1g = (W1.T.reshape(KT, KP, H).transpose(1, 0, 2)
           .reshape(KP, KT * H).astype(_BF16))
    ws = np.concatenate([Ws1, Ws2], axis=0)                # [136,128,128]
    wst = ws.transpose(0, 2, 1)                            # lhsT per layer
    wsg = (wst.reshape(NCH, CHL, H, H).transpose(0, 2, 1, 3)
           .reshape(NCH, H, CHL * H).astype(_BF16))
    bias = np.concatenate([b1[None, :],
                           np.concatenate([bs1, bs2], axis=0)],
                          axis=0).T.astype(np.float32)     # [128, 137]
    bias = np.ascontiguousarray(bias)
    woutt = np.ascontiguousarray(Wout.T).astype(_BF16)     # [128, 10]
    boutc = np.ascontiguousarray(bout.reshape(NC_OUT, 1)).astype(np.float32)

    in_maps = []
    for cid in range(N_CORES):
        xc = xf[:, cid * BC:(cid + 1) * BC]                # [784, 2048]
        xg = (xc.reshape(KT, KP, NG, GW).transpose(2, 1, 0, 3)
              .reshape(NG, KP, KT * GW).astype(_BF16))
        in_maps.append({
            "xg": np.ascontiguousarray(xg),
            "w1g": w1g, "wsg": wsg, "bias": bias,
            "woutt": woutt, "bout": boutc,
        })
    return in_maps


def kernel(**inputs):
    global _COMPILED
    _ensure_axon_hooks()
    from concourse.bass_utils import run_bass_kernel_spmd

    in_maps = _prep_inputs(
        np.asarray(inputs["x"], dtype=np.float32),
        np.asarray(inputs["W1"], dtype=np.float32),
        np.asarray(inputs["b1"], dtype=np.float32),
        np.asarray(inputs["Ws1"], dtype=np.float32),
        np.asarray(inputs["bs1"], dtype=np.float32),
        np.asarray(inputs["Ws2"], dtype=np.float32),
        np.asarray(inputs["bs2"], dtype=np.float32),
        np.asarray(inputs["Wout"], dtype=np.float32),
        np.asarray(inputs["bout"], dtype=np.float32),
    )
    if _COMPILED is None:
        _COMPILED = _build()
    try:
        res = run_bass_kernel_spmd(_COMPILED, in_maps, list(range(N_CORES)))
    except Exception:
        res = run_bass_kernel_spmd(_COMPILED, in_maps, list(range(N_CORES)))
    out = np.empty((B, NC_OUT), dtype=np.float32)
    for cid in range(N_CORES):
        out[cid * BC:(cid + 1) * BC] = res.results[cid]["out"].T
    return out


# revision 10
# speedup vs baseline: 1.2870x; 1.2870x over previous
"""Trainium2 Bass kernel for nn_AlainFC: deep narrow MLP, pure data parallel.

Reference computation (B=16384):
  x [B,1,28,28] -> flat [B,784]
  first = relu(x @ W1.T + b1)              # [B,128]
  h1 = 128x chain: h = relu(h @ Ws1[i].T + bs1[i])
  h2 = 8x chain on (first + h1) with Ws2/bs2
  out = h2 @ Wout.T + bout                 # [B,10]

Strategy: shard batch across 8 cores (2048 each), replicate weights.
Per core: activations live as [128 feat partitions, batch free] in SBUF.
Batch is split into 4 groups of 512 (one PSUM bank per matmul). bf16
matmuls (fp32 PSUM accumulate); bias+relu split across ScalarE (ACT) and
VectorE (DVE) per group so the PSUM drain is parallelized. All weights are
pre-transposed/packed/bf16-converted on host (free).
"""
import sys
import types

sys.path.insert(0, "/opt/trn_rl_repo")

import numpy as np
import ml_dtypes

N_CORES = 8
B = 16384
BC = B // N_CORES        # 2048 batch per core
H = 128
IN = 784
KT = 7                   # k-tiles for layer 0 (784 = 7*112)
KP = 112                 # partitions per k-tile
NG = 4                   # batch groups per core
GW = BC // NG            # 512 = group width (one PSUM bank)
NL = 136                 # chained 128x128 layers (128 + 8)
NCH = 17                 # weight DMA chunks
CHL = NL // NCH          # 8 layers per chunk
NC_OUT = 10

_BF16 = ml_dtypes.bfloat16
_COMPILED = None


def _ensure_axon_hooks():
    """Shim antenv.axon_hooks (missing on this image) so trace=True works."""
    try:
        import antenv
    except ImportError:
        return
    if "antenv.axon_hooks" in sys.modules:
        return
    mod = types.ModuleType("antenv.axon_hooks")
    mod._hook = None

    def set_axon_ntff_profile_hook(h):
        mod._hook = h

    def get_axon_ntff_profile_hook():
        return mod._hook

    mod.set_axon_ntff_profile_hook = set_axon_ntff_profile_hook
    mod.get_axon_ntff_profile_hook = get_axon_ntff_profile_hook
    sys.modules["antenv.axon_hooks"] = mod
    antenv.axon_hooks = mod
    try:
        from trn_agent_boot.trn_boot import _ntff_profile_via_ctypes

        hook = _ntff_profile_via_ctypes("/opt/axon/libaxon_pjrt.so")
        if hook is not None:
            set_axon_ntff_profile_hook(hook)
    except Exception:
        pass


def _build():
    import concourse.bass as bass
    import concourse.bacc as bacc
    import concourse.mybir as mybir
    from concourse import tile
    from concourse.tile_rust import add_dep_helper

    dt = mybir.dt
    AF = mybir.ActivationFunctionType
    ALU = mybir.AluOpType

    nc = bacc.Bacc("TRN2", target_bir_lowering=False, debug=False,
                   num_devices=N_CORES)

    xg_ext = nc.dram_tensor("xg", [NG, KP, KT * GW], dt.bfloat16,
                            kind="ExternalInput")
    w1g_ext = nc.dram_tensor("w1g", [KP, KT * H], dt.bfloat16,
                             kind="ExternalInput")
    wsg_ext = nc.dram_tensor("wsg", [NCH, H, CHL * H], dt.bfloat16,
                             kind="ExternalInput")
    bias_ext = nc.dram_tensor("bias", [H, NL + 1], dt.float32,
                              kind="ExternalInput")
    woutt_ext = nc.dram_tensor("woutt", [H, NC_OUT], dt.bfloat16,
                               kind="ExternalInput")
    bout_ext = nc.dram_tensor("bout", [NC_OUT, 1], dt.float32,
                              kind="ExternalInput")
    out_ext = nc.dram_tensor("out", [NC_OUT, BC], dt.float32,
                             kind="ExternalOutput")

    with tile.TileContext(nc) as tc:
        from contextlib import ExitStack

        with ExitStack() as ctx:
            const_pool = ctx.enter_context(tc.tile_pool(name="const", bufs=1))
            xg_pool = ctx.enter_context(tc.tile_pool(name="xg", bufs=NG))
            wsg_pool = ctx.enter_context(tc.tile_pool(name="wsg", bufs=NCH))
            first_pool = ctx.enter_context(tc.tile_pool(name="first", bufs=NG))
            h_pool = ctx.enter_context(tc.tile_pool(name="h", bufs=14))
            out_pool = ctx.enter_context(tc.tile_pool(name="osb", bufs=NG))
            psum_pool = ctx.enter_context(
                tc.tile_pool(name="psum", bufs=8, space="PSUM"))

            # ---- DMA inputs. DMA throughput here is descriptor-rate
            # bound per queue (~20-25 GB/s each), so spread transfers over
            # many HWDGE queues and let them fair-share; x groups first ----
            w1g_sb = const_pool.tile([KP, KT * H], dt.bfloat16, tag="w1g")
            nc.sync.dma_start(w1g_sb[:], w1g_ext.ap())
            bias_sb = const_pool.tile([H, NL + 1], dt.float32, tag="bias")
            nc.sync.dma_start(bias_sb[:], bias_ext.ap())

            xg_sb = []
            QP = KP // 4
            for g in range(NG):
                t = xg_pool.tile([KP, KT * GW], dt.bfloat16, tag="xg")
                for q in range(4):
                    nc.sync.dma_start(t[q * QP:(q + 1) * QP, :],
                                      xg_ext.ap()[g, q * QP:(q + 1) * QP])
                xg_sb.append(t)

            wsg_sb = []
            for c in range(NCH):
                t = wsg_pool.tile([H, CHL * H], dt.bfloat16, tag="wsg")
                nc.sync.dma_start(t[:], wsg_ext.ap()[c])
                wsg_sb.append(t)

            woutt_sb = const_pool.tile([H, NC_OUT], dt.bfloat16, tag="woutt")
            nc.sync.dma_start(woutt_sb[:], woutt_ext.ap())
            bout_sb = const_pool.tile([NC_OUT, 1], dt.float32, tag="bout")
            nc.sync.dma_start(bout_sb[:], bout_ext.ap())

            def relu_bias(g, psum_t, out_t, b_ap):
                # bias+relu: even groups on ScalarE, odd groups on VectorE
                # (one PSUM bank each -- parallel drain, no bank conflicts)
                if g % 2 == 0:
                    nc.scalar.activation(out_t[:], psum_t[:], AF.Relu,
                                         bias=b_ap)
                else:
                    nc.vector.tensor_scalar(out_t[:], psum_t[:], b_ap, 0.0,
                                            ALU.add, ALU.max)

            # ---- layer 0: [784 -> 128], K accumulated over 7 k-tiles ----
            h_cur = []
            for g in range(NG):
                ps = psum_pool.tile([H, GW], dt.float32, tag="ps")
                for k in range(KT):
                    nc.tensor.matmul(
                        ps[:],
                        w1g_sb[:, k * H:(k + 1) * H],
                        xg_sb[g][:, k * GW:(k + 1) * GW],
                        start=(k == 0), stop=(k == KT - 1),
                    )
                f = first_pool.tile([H, GW], dt.bfloat16, tag="first")
                relu_bias(g, ps, f, bias_sb[:, 0:1])
                h_cur.append(f)
            first_tiles = list(h_cur)

            # ---- 136 chained layers ----
            for l in range(NL):
                c, j = divmod(l, CHL)
                w_ap = wsg_sb[c][:, j * H:(j + 1) * H]
                b_ap = bias_sb[:, l + 1:l + 2]
                h_next = []
                for g in range(NG):
                    ps = psum_pool.tile([H, GW], dt.float32, tag="ps")
                    if l == 128:
                        # first Ws2 layer reads (first + h1): accumulate two
                        # matmuls instead of an explicit elementwise add
                        nc.tensor.matmul(ps[:], w_ap, h_cur[g][:],
                                         start=True, stop=False)
                        nc.tensor.matmul(ps[:], w_ap, first_tiles[g][:],
                                         start=False, stop=True)
                    else:
                        nc.tensor.matmul(ps[:], w_ap, h_cur[g][:],
                                         start=True, stop=True)
                    hn = h_pool.tile([H, GW], dt.bfloat16, tag="h")
                    relu_bias(g, ps, hn, b_ap)
                    h_next.append(hn)
                h_cur = h_next

            # ---- output layer: [128 -> 10] + bias (no relu) ----
            for g in range(NG):
                ps = psum_pool.tile([NC_OUT, GW], dt.float32, tag="ps")
                nc.tensor.matmul(ps[:], woutt_sb[:], h_cur[g][:],
                                 start=True, stop=True)
                ot = out_pool.tile([NC_OUT, GW], dt.float32, tag="ot")
                if g % 2 == 0:
                    nc.scalar.activation(ot[:], ps[:], AF.Identity,
                                         bias=bout_sb[:])
                else:
                    nc.vector.tensor_scalar(ot[:], ps[:], bout_sb[:], None,
                                            ALU.add)
                nc.sync.dma_start(out_ext.ap()[:, g * GW:(g + 1) * GW], ot[:])

    nc.compile()
    return nc


def _prep_inputs(x, W1, b1, Ws1, bs1, Ws2, bs2, Wout, bout):
    """Host-side shard + pack. Returns per-core input maps."""
    xf = np.ascontiguousarray(x.reshape(B, IN).T)          # [784, B]
    # per-core [784, 2048] -> groups [4, 112, 7*512]
    w1g = (W1.T.reshape(KT, KP, H).transpose(1, 0, 2)
           .reshape(KP, KT * H).astype(_BF16))
    ws = np.concatenate([Ws1, Ws2], axis=0)                # [136,128,128]
    wst = ws.transpose(0, 2, 1)                            # lhsT per layer
    wsg = (wst.reshape(NCH, CHL, H, H).transpose(0, 2, 1, 3)
           .reshape(NCH, H, CHL * H).astype(_BF16))
    bias = np.concatenate([b1[None, :],
                           np.concatenate([bs1, bs2], axis=0)],
                          axis=0).T.astype(np.float32)     # [128, 137]
    bias = np.ascontiguousarray(bias)
    woutt = np.ascontiguousarray(Wout.T).astype(_BF16)     # [128, 10]
    boutc = np.ascontiguousarray(bout.reshape(NC_OUT, 1)).astype(np.float32)

    in_maps = []
    for cid in range(N_CORES):
        xc = xf[:, cid * BC:(cid + 1) * BC]                # [784, 2048]
        xg = (xc.reshape(KT, KP, NG, GW).transpose(2, 1, 0, 3)
              .reshape(NG, KP, KT * GW).astype(_BF16))
        in_maps.append({
            "xg": np.ascontiguousarray(xg),
            "w1g": w1g, "wsg": wsg, "bias": bias,
            "woutt": woutt, "bout": boutc,
        })
    return in_maps


def kernel(**inputs):
    global _COMPILED
    _ensure_axon_hooks()
    from concourse.bass_utils import run_bass_kernel_spmd

    in_maps = _prep_inputs(
        np.asarray(inputs["x"], dtype=np.float32),
        np.asarray(inputs["W1"], dtype=np.float32),
        np.asarray(inputs["b1"], dtype=np.float32),
        np.asarray(inputs["Ws1"], dtype=np.float32),
        np.asarray(inputs["bs1"], dtype=np.float32),
        np.asarray(inputs["Ws2"], dtype=np.float32),
        np.asarray(inputs["bs2"], dtype=np.float32),
        np.asarray(inputs["Wout"], dtype=np.float32),
        np.asarray(inputs["bout"], dtype=np.float32),
    )
    if _COMPILED is None:
        _COMPILED = _build()
    try:
        res = run_bass_kernel_spmd(_COMPILED, in_maps, list(range(N_CORES)))
    except Exception:
        res = run_bass_kernel_spmd(_COMPILED, in_maps, list(range(N_CORES)))
    out = np.empty((B, NC_OUT), dtype=np.float32)
    for cid in range(N_CORES):
        out[cid * BC:(cid + 1) * BC] = res.results[cid]["out"].T
    return out


# revision 11
# speedup vs baseline: 1.2905x; 1.0027x over previous
"""Trainium2 Bass kernel for nn_AlainFC: deep narrow MLP, pure data parallel.

Reference computation (B=16384):
  x [B,1,28,28] -> flat [B,784]
  first = relu(x @ W1.T + b1)              # [B,128]
  h1 = 128x chain: h = relu(h @ Ws1[i].T + bs1[i])
  h2 = 8x chain on (first + h1) with Ws2/bs2
  out = h2 @ Wout.T + bout                 # [B,10]

Strategy: shard batch across 8 cores (2048 each), replicate weights.
Per core: activations live as [128 feat partitions, batch free] in SBUF.
Batch is split into 4 groups of 512 (one PSUM bank per matmul). bf16
matmuls (fp32 PSUM accumulate); bias+relu split across ScalarE (ACT) and
VectorE (DVE) per group so the PSUM drain is parallelized. All weights are
pre-transposed/packed/bf16-converted on host (free).
"""
import sys
import types

sys.path.insert(0, "/opt/trn_rl_repo")

import numpy as np
import ml_dtypes

N_CORES = 8
B = 16384
BC = B // N_CORES        # 2048 batch per core
H = 128
IN = 784
KT = 7                   # k-tiles for layer 0 (784 = 7*112)
KP = 112                 # partitions per k-tile
NG = 4                   # batch groups per core
GW = BC // NG            # 512 = group width (one PSUM bank)
NL = 136                 # chained 128x128 layers (128 + 8)
NCH = 17                 # weight DMA chunks
CHL = NL // NCH          # 8 layers per chunk
NC_OUT = 10

_BF16 = ml_dtypes.bfloat16
_COMPILED = None


def _ensure_axon_hooks():
    """Shim antenv.axon_hooks (missing on this image) so trace=True works."""
    try:
        import antenv
    except ImportError:
        return
    if "antenv.axon_hooks" in sys.modules:
        return
    mod = types.ModuleType("antenv.axon_hooks")
    mod._hook = None

    def set_axon_ntff_profile_hook(h):
        mod._hook = h

    def get_axon_ntff_profile_hook():
        return mod._hook

    mod.set_axon_ntff_profile_hook = set_axon_ntff_profile_hook
    mod.get_axon_ntff_profile_hook = get_axon_ntff_profile_hook
    sys.modules["antenv.axon_hooks"] = mod
    antenv.axon_hooks = mod
    try:
        from trn_agent_boot.trn_boot import _ntff_profile_via_ctypes

        hook = _ntff_profile_via_ctypes("/opt/axon/libaxon_pjrt.so")
        if hook is not None:
            set_axon_ntff_profile_hook(hook)
    except Exception:
        pass


def _build():
    import concourse.bass as bass
    import concourse.bacc as bacc
    import concourse.mybir as mybir
    from concourse import tile
    from concourse.tile_rust import add_dep_helper

    dt = mybir.dt
    AF = mybir.ActivationFunctionType
    ALU = mybir.AluOpType

    nc = bacc.Bacc("TRN2", target_bir_lowering=False, debug=False,
                   num_devices=N_CORES)

    xg_ext = nc.dram_tensor("xg", [NG, KP, KT * GW], dt.bfloat16,
                            kind="ExternalInput")
    w1g_ext = nc.dram_tensor("w1g", [KP, KT * H], dt.bfloat16,
                             kind="ExternalInput")
    wsg_ext = nc.dram_tensor("wsg", [NCH, H, CHL * H], dt.bfloat16,
                             kind="ExternalInput")
    bias_ext = nc.dram_tensor("bias", [H, NL + 1], dt.float32,
                              kind="ExternalInput")
    woutt_ext = nc.dram_tensor("woutt", [H, NC_OUT], dt.bfloat16,
                               kind="ExternalInput")
    bout_ext = nc.dram_tensor("bout", [NC_OUT, 1], dt.float32,
                              kind="ExternalInput")
    out_ext = nc.dram_tensor("out", [NC_OUT, BC], dt.float32,
                             kind="ExternalOutput")

    with tile.TileContext(nc) as tc:
        from contextlib import ExitStack

        with ExitStack() as ctx:
            const_pool = ctx.enter_context(tc.tile_pool(name="const", bufs=1))
            xg_pool = ctx.enter_context(tc.tile_pool(name="xg", bufs=NG))
            wsg_pool = ctx.enter_context(tc.tile_pool(name="wsg", bufs=NCH))
            first_pool = ctx.enter_context(tc.tile_pool(name="first", bufs=NG))
            h_pool = ctx.enter_context(tc.tile_pool(name="h", bufs=14))
            out_pool = ctx.enter_context(tc.tile_pool(name="osb", bufs=NG))
            psum_pool = ctx.enter_context(
                tc.tile_pool(name="psum", bufs=8, space="PSUM"))

            # ---- DMA inputs. DMA throughput here is descriptor-rate
            # bound per queue (~20-25 GB/s each), so spread transfers over
            # many HWDGE queues and let them fair-share; x groups first ----
            w1g_sb = const_pool.tile([KP, KT * H], dt.bfloat16, tag="w1g")
            nc.sync.dma_start(w1g_sb[:], w1g_ext.ap())
            bias_sb = const_pool.tile([H, NL + 1], dt.float32, tag="bias")
            nc.sync.dma_start(bias_sb[:], bias_ext.ap())

            xg_sb = []
            HP = KP // 2
            for g in range(NG):
                t = xg_pool.tile([KP, KT * GW], dt.bfloat16, tag="xg")
                nc.sync.dma_start(t[0:HP, :], xg_ext.ap()[g, 0:HP])
                nc.sync.dma_start(t[HP:KP, :], xg_ext.ap()[g, HP:KP])
                xg_sb.append(t)

            wsg_sb = []
            for c in range(NCH):
                t = wsg_pool.tile([H, CHL * H], dt.bfloat16, tag="wsg")
                nc.sync.dma_start(t[:], wsg_ext.ap()[c])
                wsg_sb.append(t)

            woutt_sb = const_pool.tile([H, NC_OUT], dt.bfloat16, tag="woutt")
            nc.sync.dma_start(woutt_sb[:], woutt_ext.ap())
            bout_sb = const_pool.tile([NC_OUT, 1], dt.float32, tag="bout")
            nc.sync.dma_start(bout_sb[:], bout_ext.ap())

            def relu_bias(g, psum_t, out_t, b_ap):
                # bias+relu: even groups on ScalarE, odd groups on VectorE
                # (one PSUM bank each -- parallel drain, no bank conflicts)
                if g % 2 == 0:
                    nc.scalar.activation(out_t[:], psum_t[:], AF.Relu,
                                         bias=b_ap)
                else:
                    nc.vector.tensor_scalar(out_t[:], psum_t[:], b_ap, 0.0,
                                            ALU.add, ALU.max)

            # ---- layer 0: [784 -> 128], K accumulated over 7 k-tiles ----
            h_cur = []
            for g in range(NG):
                ps = psum_pool.tile([H, GW], dt.float32, tag="ps")
                for k in range(KT):
                    nc.tensor.matmul(
                        ps[:],
                        w1g_sb[:, k * H:(k + 1) * H],
                        xg_sb[g][:, k * GW:(k + 1) * GW],
                        start=(k == 0), stop=(k == KT - 1),
                    )
                f = first_pool.tile([H, GW], dt.bfloat16, tag="first")
                relu_bias(g, ps, f, bias_sb[:, 0:1])
                h_cur.append(f)
            first_tiles = list(h_cur)

            # ---- 136 chained layers ----
            for l in range(NL):
                c, j = divmod(l, CHL)
                w_ap = wsg_sb[c][:, j * H:(j + 1) * H]
                b_ap = bias_sb[:, l + 1:l + 2]
                h_next = []
                for g in range(NG):
                    ps = psum_pool.tile([H, GW], dt.float32, tag="ps")
                    if l == 128:
                        # first Ws2 layer reads (first + h1): accumulate two
                        # matmuls instead of an explicit elementwise add
                        nc.tensor.matmul(ps[:], w_ap, h_cur[g][:],
                                         start=True, stop=False)
                        nc.tensor.matmul(ps[:], w_ap, first_tiles[g][:],
                                         start=False, stop=True)
                    else:
                        nc.tensor.matmul(ps[:], w_ap, h_cur[g][:],
                                         start=True, stop=True)
                    hn = h_pool.tile([H, GW], dt.bfloat16, tag="h")
                    relu_bias(g, ps, hn, b_ap)
                    h_next.append(hn)
                h_cur = h_next

            # ---- output layer: [128 -> 10] + bias (no relu) ----
            for g in range(NG):
                ps = psum_pool.tile([NC_OUT, GW], dt.float32, tag="ps")
                nc.tensor.matmul(ps[:], woutt_sb[:], h_cur[g][:],
                                 start=True, stop=True)
                ot = out_pool.tile([NC_OUT, GW], dt.float32, tag="ot")
                if g % 2 == 0:
                    nc.scalar.activation(ot[:], ps[:], AF.Identity,
                                         bias=bout_sb[:])
                else:
                    nc.vector.tensor_scalar(ot[:], ps[:], bout_sb[:], None,
                                            ALU.add)
                nc.sync.dma_start(out_ext.ap()[:, g * GW:(g + 1) * GW], ot[:])

    nc.compile()
    return nc


def _prep_inputs(x, W1, b1, Ws1, bs1, Ws2, bs2, Wout, bout):
    """Host-side shard + pack. Returns per-core input maps."""
    xf = np.ascontiguousarray(x.reshape(B, IN).T)          # [784, B]
    # per-core [784, 2048] -> groups [4, 112, 7*512]
    w1g = (W1.T.reshape(KT, KP, H).transpose(1, 0, 2)
           .reshape(KP, KT * H).astype(_BF16))
    ws = np.concatenate([Ws1, Ws2], axis=0)                # [136,128,128]
    wst = ws.transpose(0, 2, 1)                            # lhsT per layer
    wsg = (wst.reshape(NCH, CHL, H, H).transpose(0, 2, 1, 3)
           .reshape(NCH, H, CHL * H).astype(_BF16))
    bias = np.concatenate([b1[None, :],
                           np.concatenate([bs1, bs2], axis=0)],
                          axis=0).T.astype(np.float32)     # [128, 137]
    bias = np.ascontiguousarray(bias)
    woutt = np.ascontiguousarray(Wout.T).astype(_BF16)     # [128, 10]
    boutc = np.ascontiguousarray(bout.reshape(NC_OUT, 1)).astype(np.float32)

    in_maps = []
    for cid in range(N_CORES):
        xc = xf[:, cid * BC:(cid + 1) * BC]                # [784, 2048]
        xg = (xc.reshape(KT, KP, NG, GW).transpose(2, 1, 0, 3)
              .reshape(NG, KP, KT * GW).astype(_BF16))
        in_maps.append({
            "xg": np.ascontiguousarray(xg),
            "w1g": w1g, "wsg": wsg, "bias": bias,
            "woutt": woutt, "bout": boutc,
        })
    return in_maps


def kernel(**inputs):
    global _COMPILED
    _ensure_axon_hooks()
    from concourse.bass_utils import run_bass_kernel_spmd

    in_maps = _prep_inputs(
        np.asarray(inputs["x"], dtype=np.float32),
        np.asarray(inputs["W1"], dtype=np.float32),
        np.asarray(inputs["b1"], dtype=np.float32),
        np.asarray(inputs["Ws1"], dtype=np.float32),
        np.asarray(inputs["bs1"], dtype=np.float32),
        np.asarray(inputs["Ws2"], dtype=np.float32),
        np.asarray(inputs["bs2"], dtype=np.float32),
        np.asarray(inputs["Wout"], dtype=np.float32),
        np.asarray(inputs["bout"], dtype=np.float32),
    )
    if _COMPILED is None:
        _COMPILED = _build()
    try:
        res = run_bass_kernel_spmd(_COMPILED, in_maps, list(range(N_CORES)))
    except Exception:
        res = run_bass_kernel_spmd(_COMPILED, in_maps, list(range(N_CORES)))
    out = np.empty((B, NC_OUT), dtype=np.float32)
    for cid in range(N_CORES):
        out[cid * BC:(cid + 1) * BC] = res.results[cid]["out"].T
    return out


# revision 12
# speedup vs baseline: 1.3160x; 1.0197x over previous
"""Trainium2 Bass kernel for nn_AlainFC: deep narrow MLP, pure data parallel.

Reference computation (B=16384):
  x [B,1,28,28] -> flat [B,784]
  first = relu(x @ W1.T + b1)              # [B,128]
  h1 = 128x chain: h = relu(h @ Ws1[i].T + bs1[i])
  h2 = 8x chain on (first + h1) with Ws2/bs2
  out = h2 @ Wout.T + bout                 # [B,10]

Strategy: shard batch across 8 cores (2048 each), replicate weights.
Per core: activations live as [128 feat partitions, batch free] in SBUF.
Batch is split into 4 groups of 512 (one PSUM bank per matmul). bf16
matmuls (fp32 PSUM accumulate); bias+relu split across ScalarE (ACT) and
VectorE (DVE) per group so the PSUM drain is parallelized. All weights are
pre-transposed/packed/bf16-converted on host (free).
"""
import sys
import types

sys.path.insert(0, "/opt/trn_rl_repo")

import numpy as np
import ml_dtypes

N_CORES = 8
B = 16384
BC = B // N_CORES        # 2048 batch per core
H = 128
IN = 784
KT = 7                   # k-tiles for layer 0 (784 = 7*112)
KP = 112                 # partitions per k-tile
NG = 4                   # batch groups per core
GW = BC // NG            # 512 = group width (one PSUM bank)
NL = 136                 # chained 128x128 layers (128 + 8)
NCH = 17                 # weight DMA chunks
CHL = NL // NCH          # 8 layers per chunk
NC_OUT = 10

_BF16 = ml_dtypes.bfloat16
_COMPILED = None


def _ensure_axon_hooks():
    """Shim antenv.axon_hooks (missing on this image) so trace=True works."""
    try:
        import antenv
    except ImportError:
        return
    if "antenv.axon_hooks" in sys.modules:
        return
    mod = types.ModuleType("antenv.axon_hooks")
    mod._hook = None

    def set_axon_ntff_profile_hook(h):
        mod._hook = h

    def get_axon_ntff_profile_hook():
        return mod._hook

    mod.set_axon_ntff_profile_hook = set_axon_ntff_profile_hook
    mod.get_axon_ntff_profile_hook = get_axon_ntff_profile_hook
    sys.modules["antenv.axon_hooks"] = mod
    antenv.axon_hooks = mod
    try:
        from trn_agent_boot.trn_boot import _ntff_profile_via_ctypes

        hook = _ntff_profile_via_ctypes("/opt/axon/libaxon_pjrt.so")
        if hook is not None:
            set_axon_ntff_profile_hook(hook)
    except Exception:
        pass


def _build():
    import concourse.bass as bass
    import concourse.bacc as bacc
    import concourse.mybir as mybir
    from concourse import tile
    from concourse.tile_rust import add_dep_helper

    dt = mybir.dt
    AF = mybir.ActivationFunctionType
    ALU = mybir.AluOpType

    nc = bacc.Bacc("TRN2", target_bir_lowering=False, debug=False,
                   num_devices=N_CORES)

    xg_ext = nc.dram_tensor("xg", [NG, KP, KT * GW], dt.bfloat16,
                            kind="ExternalInput")
    w1g_ext = nc.dram_tensor("w1g", [KP, KT * H], dt.bfloat16,
                             kind="ExternalInput")
    wsg_ext = nc.dram_tensor("wsg", [NCH, H, CHL * H], dt.bfloat16,
                             kind="ExternalInput")
    bias_ext = nc.dram_tensor("bias", [H, NL + 1], dt.float32,
                              kind="ExternalInput")
    woutt_ext = nc.dram_tensor("woutt", [H, NC_OUT], dt.bfloat16,
                               kind="ExternalInput")
    bout_ext = nc.dram_tensor("bout", [NC_OUT, 1], dt.float32,
                              kind="ExternalInput")
    out_ext = nc.dram_tensor("out", [NC_OUT, BC], dt.float32,
                             kind="ExternalOutput")

    with tile.TileContext(nc) as tc:
        from contextlib import ExitStack

        with ExitStack() as ctx:
            const_pool = ctx.enter_context(tc.tile_pool(name="const", bufs=1))
            xg_pool = ctx.enter_context(tc.tile_pool(name="xg", bufs=NG))
            wsg_pool = ctx.enter_context(tc.tile_pool(name="wsg", bufs=NCH))
            first_pool = ctx.enter_context(tc.tile_pool(name="first", bufs=NG))
            h_pool = ctx.enter_context(tc.tile_pool(name="h", bufs=14))
            out_pool = ctx.enter_context(tc.tile_pool(name="osb", bufs=NG))
            psum_pool = ctx.enter_context(
                tc.tile_pool(name="psum", bufs=8, space="PSUM"))

            # ---- DMA inputs. DMA throughput here is descriptor-rate
            # bound per queue (~20-25 GB/s each), so spread transfers over
            # many HWDGE queues and let them fair-share; x groups first ----
            w1g_sb = const_pool.tile([KP, KT * H], dt.bfloat16, tag="w1g")
            nc.sync.dma_start(w1g_sb[:], w1g_ext.ap())
            bias_sb = const_pool.tile([H, NL + 1], dt.float32, tag="bias")
            nc.sync.dma_start(bias_sb[:], bias_ext.ap())

            xg_sb = []
            CW = KT * GW // 2
            for g in range(NG):
                t = xg_pool.tile([KP, KT * GW], dt.bfloat16, tag="xg")
                nc.sync.dma_start(t[:, 0:CW], xg_ext.ap()[g][:, 0:CW])
                nc.sync.dma_start(t[:, CW:], xg_ext.ap()[g][:, CW:])
                xg_sb.append(t)

            wsg_sb = []
            for c in range(NCH):
                t = wsg_pool.tile([H, CHL * H], dt.bfloat16, tag="wsg")
                nc.sync.dma_start(t[:], wsg_ext.ap()[c])
                wsg_sb.append(t)

            woutt_sb = const_pool.tile([H, NC_OUT], dt.bfloat16, tag="woutt")
            nc.sync.dma_start(woutt_sb[:], woutt_ext.ap())
            bout_sb = const_pool.tile([NC_OUT, 1], dt.float32, tag="bout")
            nc.sync.dma_start(bout_sb[:], bout_ext.ap())

            def relu_bias(g, psum_t, out_t, b_ap):
                # bias+relu: even groups on ScalarE, odd groups on VectorE
                # (one PSUM bank each -- parallel drain, no bank conflicts)
                if g % 2 == 0:
                    nc.scalar.activation(out_t[:], psum_t[:], AF.Relu,
                                         bias=b_ap)
                else:
                    nc.vector.tensor_scalar(out_t[:], psum_t[:], b_ap, 0.0,
                                            ALU.add, ALU.max)

            # ---- layer 0: [784 -> 128], K accumulated over 7 k-tiles ----
            h_cur = []
            for g in range(NG):
                ps = psum_pool.tile([H, GW], dt.float32, tag="ps")
                for k in range(KT):
                    nc.tensor.matmul(
                        ps[:],
                        w1g_sb[:, k * H:(k + 1) * H],
                        xg_sb[g][:, k * GW:(k + 1) * GW],
                        start=(k == 0), stop=(k == KT - 1),
                    )
                f = first_pool.tile([H, GW], dt.bfloat16, tag="first")
                relu_bias(g, ps, f, bias_sb[:, 0:1])
                h_cur.append(f)
            first_tiles = list(h_cur)

            # ---- 136 chained layers ----
            for l in range(NL):
                c, j = divmod(l, CHL)
                w_ap = wsg_sb[c][:, j * H:(j + 1) * H]
                b_ap = bias_sb[:, l + 1:l + 2]
                h_next = []
                for g in range(NG):
                    ps = psum_pool.tile([H, GW], dt.float32, tag="ps")
                    if l == 128:
                        # first Ws2 layer reads (first + h1): accumulate two
                        # matmuls instead of an explicit elementwise add
                        nc.tensor.matmul(ps[:], w_ap, h_cur[g][:],
                                         start=True, stop=False)
                        nc.tensor.matmul(ps[:], w_ap, first_tiles[g][:],
                                         start=False, stop=True)
                    else:
                        nc.tensor.matmul(ps[:], w_ap, h_cur[g][:],
                                         start=True, stop=True)
                    hn = h_pool.tile([H, GW], dt.bfloat16, tag="h")
                    relu_bias(g, ps, hn, b_ap)
                    h_next.append(hn)
                h_cur = h_next

            # ---- output layer: [128 -> 10] + bias (no relu) ----
            for g in range(NG):
                ps = psum_pool.tile([NC_OUT, GW], dt.float32, tag="ps")
                nc.tensor.matmul(ps[:], woutt_sb[:], h_cur[g][:],
                                 start=True, stop=True)
                ot = out_pool.tile([NC_OUT, GW], dt.float32, tag="ot")
                if g % 2 == 0:
                    nc.scalar.activation(ot[:], ps[:], AF.Identity,
                                         bias=bout_sb[:])
                else:
                    nc.vector.tensor_scalar(ot[:], ps[:], bout_sb[:], None,
                                            ALU.add)
                nc.sync.dma_start(out_ext.ap()[:, g * GW:(g + 1) * GW], ot[:])

    nc.compile()
    return nc


def _prep_inputs(x, W1, b1, Ws1, bs1, Ws2, bs2, Wout, bout):
    """Host-side shard + pack. Returns per-core input maps."""
    xf = np.ascontiguousarray(x.reshape(B, IN).T)          # [784, B]
    # per-core [784, 2048] -> groups [4, 112, 7*512]
    w1g = (W1.T.reshape(KT, KP, H).transpose(1, 0, 2)
           .reshape(KP, KT * H).astype(_BF16))
    ws = np.concatenate([Ws1, Ws2], axis=0)                # [136,128,128]
    wst = ws.transpose(0, 2, 1)                            # lhsT per layer
    wsg = (wst.reshape(NCH, CHL, H, H).transpose(0, 2, 1, 3)
           .reshape(NCH, H, CHL * H).astype(_BF16))
    bias = np.concatenate([b1[None, :],
                           np.concatenate([bs1, bs2], axis=0)],
                          axis=0).T.astype(np.float32)     # [128, 137]
    bias = np.ascontiguousarray(bias)
    woutt = np.ascontiguousarray(Wout.T).astype(_BF16)     # [128, 10]
    boutc = np.ascontiguousarray(bout.reshape(NC_OUT, 1)).astype(np.float32)

    in_maps = []
    for cid in range(N_CORES):
        xc = xf[:, cid * BC:(cid + 1) * BC]                # [784, 2048]
        xg = (xc.reshape(KT, KP, NG, GW).transpose(2, 1, 0, 3)
              .reshape(NG, KP, KT * GW).astype(_BF16))
        in_maps.append({
            "xg": np.ascontiguousarray(xg),
            "w1g": w1g, "wsg": wsg, "bias": bias,
            "woutt": woutt, "bout": boutc,
        })
    return in_maps


def kernel(**inputs):
    global _COMPILED
    _ensure_axon_hooks()
    from concourse.bass_utils import run_bass_kernel_spmd

    in_maps = _prep_inputs(
        np.asarray(inputs["x"], dtype=np.float32),
        np.asarray(inputs["W1"], dtype=np.float32),
        np.asarray(inputs["b1"], dtype=np.float32),
        np.asarray(inputs["Ws1"], dtype=np.float32),
        np.asarray(inputs["bs1"], dtype=np.float32),
        np.asarray(inputs["Ws2"], dtype=np.float32),
        np.asarray(inputs["bs2"], dtype=np.float32),
        np.asarray(inputs["Wout"], dtype=np.float32),
        np.asarray(inputs["bout"], dtype=np.float32),
    )
    if _COMPILED is None:
        _COMPILED = _build()
    try:
        res = run_bass_kernel_spmd(_COMPILED, in_maps, list(range(N_CORES)))
    except Exception:
        res = run_bass_kernel_spmd(_COMPILED, in_maps, list(range(N_CORES)))
    out = np.empty((B, NC_OUT), dtype=np.float32)
    for cid in range(N_CORES):
        out[cid * BC:(cid + 1) * BC] = res.results[cid]["out"].T
    return out
